# revision 17
# baseline (speedup 1.0000x reference)
"""M2M-GNN (nn_M2MGNNPro) Trainium2 kernel, 8-core SPMD, v2.

Strategy (edge-parallel, destination-sharded, bf16 data path):
- Nodes padded to NP=50176, sharded 6272/core; per-core inputs rotated so the
  own shard occupies rows 0..6271 (identical SPMD program on all cores).
- Phase A (replicated): h0 = relu(x@W1.T+b1), ego = LN(h0), h = ego@Wlin.T
  written to a bf16 DRAM table with rows padded to 128 cols (256B for gather).
  Batched in groups of 4 node-tiles; LN stats via segmented reduces.
- Phase B (edge phase): edges sorted by dest window, split into col<32768 /
  col>=32768 gather streams (int16 idx). BOTH h[col] and h[row] fetched via
  gpsimd.dma_gather in CALL-slot chunks. Per chunk (batched across 6 tiles):
  tt = 0.5*hr+hc, d = wd.relu(tt) (seg-reduce), att = sigmoid(d), and
  xj = att*hc written into the gather buffer's pad half so each 128-edge tile
  scatters with ONE matmul: psum += S_t.T @ [hc | xj], S built per window by a
  single batched is_equal against iota. agg = [xj_sum | hc_sum - xj_sum].
- Phase C: relu/LN/blend (0.5 folded into W2) + GEMM, batched 7 tiles/group.
"""
import numpy as np

N = 50000
E = 800000
IN = 128
HID = 64
C = 2
HC = 128
OUT = 40
BETA = 0.5
TEMP = 1.0
EPS = 1e-5

NCORES = 8
P = 128
NP = 50176            # 392 tiles of 128
SH = NP // NCORES     # 6272 nodes/core, 49 windows
NWIN = SH // P        # 49
NT = NP // P          # 392
SPLIT = 32768         # int16-safe col split
CALL = 768            # gather slots per dma_gather call (ring-safe)
CHUNK = 1536          # slots per batched-math chunk (2 gather calls)
CT = CHUNK // P       # tiles per chunk
GA = 4                # phase A tiles per group
GC = 4                # phase C tiles per group

_cache = {}


def _bf16():
    import concourse.mybir as mybir
    return mybir.dt.np(mybir.dt.bfloat16)


def _host_prep(x, edge_index, W1, b1, Wlin, Watt, W2, b2):
    bf16 = _bf16()
    x = np.asarray(x, np.float32)
    row = np.asarray(edge_index[0], np.int64)
    col = np.asarray(edge_index[1], np.int64)

    x_pad = np.zeros((NP, IN), np.float32)
    x_pad[:N] = x

    core = row // SH
    meta_TA = np.zeros(NWIN, np.int64)
    meta_TB = np.zeros(NWIN, np.int64)
    percore = []
    for k in range(NCORES):
        m = core == k
        rk = row[m] - k * SH          # local dest 0..SH-1
        ck = (col[m] - k * SH) % NP   # rotated col index
        w = rk // P
        groups = []
        for wi in range(NWIN):
            mw = w == wi
            cw, rw, rl = ck[mw], rk[mw] % P, rk[mw]
            a = cw < SPLIT
            groups.append(
                ((cw[a], rw[a], rl[a]), (cw[~a] - SPLIT, rw[~a], rl[~a]))
            )
        percore.append(groups)
    for wi in range(NWIN):
        meta_TA[wi] = max(
            -(-len(percore[k][wi][0][0]) // P) for k in range(NCORES)
        )
        meta_TB[wi] = max(
            -(-len(percore[k][wi][1][0]) // P) for k in range(NCORES)
        )
    T_A, T_B = meta_TA, meta_TB
    SA = int(T_A.sum()) * P
    SB = int(T_B.sum()) * P

    def wrap16(a):
        n = len(a)
        pad = (-n) % 16
        a = np.concatenate([a, np.zeros(pad, np.int16)])
        return np.tile(a.reshape(-1, 16).T, (8, 1))

    def tilecols(a):
        # slot i=(t*128+p) -> [128, ntiles] col-per-tile layout
        return a.reshape(-1, P).T.copy()

    in_maps = []
    for k in range(NCORES):
        colA = np.zeros(SA, np.int16)
        rowA = np.zeros(SA, np.int16)
        rdA = np.full(SA, 200.0, np.float32)
        colB = np.zeros(SB, np.int16)
        rowB = np.zeros(SB, np.int16)
        rdB = np.full(SB, 200.0, np.float32)
        oa = ob = 0
        for wi in range(NWIN):
            (ca, ra, la), (cb, rb, lb) = percore[k][wi]
            na, nb = len(ca), len(cb)
            colA[oa : oa + na] = ca.astype(np.int16)
            rowA[oa : oa + na] = la.astype(np.int16)
            rdA[oa : oa + na] = ra.astype(np.float32)
            colB[ob : ob + nb] = cb.astype(np.int16)
            rowB[ob : ob + nb] = lb.astype(np.int16)
            rdB[ob : ob + nb] = rb.astype(np.float32)
            oa += int(T_A[wi]) * P
            ob += int(T_B[wi]) * P

        xk = np.roll(x_pad, -k * SH, axis=0)
        in_maps.append(
            {
                "xT": xk.T.astype(bf16).copy(),
                "colA": wrap16(colA),
                "colB": wrap16(colB),
                "rowA": wrap16(rowA),
                "rowB": wrap16(rowB),
                "rdA": tilecols(rdA).astype(bf16),
                "rdB": tilecols(rdB).astype(bf16),
            }
        )
    wd = (np.asarray(Watt[0]) - np.asarray(Watt[1])).astype(np.float32)
    shared = {
        "w1t": np.asarray(W1, np.float32).T.astype(bf16).copy(),   # [IN, HC]
        "b1row": np.asarray(b1, np.float32)[None, :].astype(bf16), # [1, HC]
        "wlint": np.asarray(Wlin, np.float32).T.astype(bf16).copy(),  # [HC, HID]
        "wdrep": np.tile(wd[None, :], (P, 1)).astype(bf16),        # [P, HID]
        "iotac": np.tile(
            np.arange(P, dtype=np.float32)[None, :], (P, 1)
        ).astype(bf16),                                            # [P, P]
        "w2t": ((1.0 - BETA) * np.asarray(W2, np.float32).T).astype(bf16).copy(),
        "b2row": np.asarray(b2, np.float32)[None, :].astype(bf16), # [1, OUT]
    }
    for im in in_maps:
        im.update(shared)
    return in_maps, (tuple(T_A.tolist()), tuple(T_B.tolist()))


def _build(T_A, T_B, reps=1):
    import concourse.bacc as bacc
    import concourse.mybir as mybir
    import concourse.tile as tile
    from concourse.library_config import mlp
    from concourse.masks import make_identity

    f32 = mybir.dt.float32
    bf16 = mybir.dt.bfloat16
    i16 = mybir.dt.int16
    Alu = mybir.AluOpType
    Act = mybir.ActivationFunctionType
    AxX = mybir.AxisListType.X

    SA = sum(T_A) * P
    SB = sum(T_B) * P
    NCHA = -(-SA // CHUNK)
    NCHB = -(-SB // CHUNK)

    nc = bacc.Bacc("TRN2")
    xT = nc.dram_tensor("xT", [IN, NP], bf16, kind="ExternalInput")
    colA = nc.dram_tensor("colA", [P, (SA + 15) // 16], i16, kind="ExternalInput")
    colB = nc.dram_tensor("colB", [P, (SB + 15) // 16], i16, kind="ExternalInput")
    rowA = nc.dram_tensor("rowA", [P, (SA + 15) // 16], i16, kind="ExternalInput")
    rowB = nc.dram_tensor("rowB", [P, (SB + 15) // 16], i16, kind="ExternalInput")
    rdA = nc.dram_tensor("rdA", [P, SA // P], bf16, kind="ExternalInput")
    rdB = nc.dram_tensor("rdB", [P, SB // P], bf16, kind="ExternalInput")
    w1t = nc.dram_tensor("w1t", [IN, HC], bf16, kind="ExternalInput")
    b1row = nc.dram_tensor("b1row", [1, HC], bf16, kind="ExternalInput")
    wlint = nc.dram_tensor("wlint", [HC, HID], bf16, kind="ExternalInput")
    wdrep = nc.dram_tensor("wdrep", [P, HID], bf16, kind="ExternalInput")
    iotac = nc.dram_tensor("iotac", [P, P], bf16, kind="ExternalInput")
    w2t = nc.dram_tensor("w2t", [HC, OUT], bf16, kind="ExternalInput")
    b2row = nc.dram_tensor("b2row", [1, OUT], bf16, kind="ExternalInput")
    hdram = nc.dram_tensor("hdram", [NP, HC], bf16)
    outd = nc.dram_tensor("out", [SH, OUT], f32, kind="ExternalOutput")

    with tile.TileContext(nc) as tc:
        with (
            tc.tile_pool(name="const", bufs=1) as cp,
            tc.tile_pool(name="work", bufs=3) as wp,
            tc.tile_pool(name="sm", bufs=3) as smp,
            tc.tile_pool(name="gather", bufs=4) as gp,
            tc.tile_pool(name="swin", bufs=3) as swp,
            tc.tile_pool(name="psA", bufs=2, space="PSUM") as psA_pool,
            tc.tile_pool(name="psT", bufs=2, space="PSUM") as psT_pool,
            tc.tile_pool(name="psQ", bufs=2, space="PSUM") as psQ_pool,
            tc.tile_pool(name="acc", bufs=2, space="PSUM") as accp,
        ):
            nc.gpsimd.load_library(mlp)
            # ---- constants to SBUF ----
            w1t_sb = cp.tile([IN, HC], bf16, tag="w1t")
            b1_sb = cp.tile([1, HC], bf16, tag="b1")
            wlint_sb = cp.tile([HC, HID], bf16, tag="wlt")
            wd_sb = cp.tile([P, HID], bf16, tag="wd")
            iota_sb = cp.tile([P, P], bf16, tag="iota")
            w2t_sb = cp.tile([HC, OUT], bf16, tag="w2t")
            b2_sb = cp.tile([1, OUT], bf16, tag="b2")
            colA_sb = cp.tile([P, (SA + 15) // 16], i16, tag="colA")
            colB_sb = cp.tile([P, (SB + 15) // 16], i16, tag="colB")
            rowA_sb = cp.tile([P, (SA + 15) // 16], i16, tag="rowA")
            rowB_sb = cp.tile([P, (SB + 15) // 16], i16, tag="rowB")
            rdA_sb = cp.tile([P, SA // P], bf16, tag="rdA")
            rdB_sb = cp.tile([P, SB // P], bf16, tag="rdB")
            for sb, dr in (
                (w1t_sb, w1t), (b1_sb, b1row), (wlint_sb, wlint),
                (wd_sb, wdrep), (iota_sb, iotac), (w2t_sb, w2t),
                (b2_sb, b2row), (colA_sb, colA), (colB_sb, colB),
                (rowA_sb, rowA), (rowB_sb, rowB),
                (rdA_sb, rdA), (rdB_sb, rdB),
            ):
                nc.sync.dma_start(sb[:], dr[:])
            ident = cp.tile([P, P], bf16, tag="ident")
            make_identity(nc, ident[:])
            ones1 = cp.tile([1, P], bf16, tag="ones1")
            nc.vector.memset(ones1[:], 1.0)
            eps_sb = cp.tile([P, 1], f32, tag="eps")
            nc.vector.memset(eps_sb[:], EPS)
            ego_sb = cp.tile([P, NWIN, HC], bf16, tag="ego")
            agg_sb = cp.tile([P, NWIN, HC], bf16, tag="agg")

            for rep in range(reps):
                tc.strict_bb_all_engine_barrier()
                # ================= Phase A =================
                for g in range(NT // GA):
                    g0 = g * GA
                    xt_t = wp.tile([IN, GA * P], bf16, tag="xt")
                    nc.sync.dma_start(xt_t[:], xT[:, g0 * P : (g0 + GA) * P])
                    psA = psA_pool.tile([P, GA, HC], f32, tag="psA")
                    for i in range(GA):
                        nc.tensor.matmul(out=psA[:, i, :],
                                         lhsT=xt_t[:, i * P : (i + 1) * P],
                                         rhs=w1t_sb[:], start=True, stop=False)
                        nc.tensor.matmul(out=psA[:, i, :], lhsT=ones1[:],
                                         rhs=b1_sb[:], start=False, stop=True)
                    r = wp.tile([P, GA, HC], bf16, tag="r")
                    nc.scalar.activation(r[:], psA[:], Act.Relu)
                    rsum = smp.tile([P, GA], f32, tag="rsum")
                    nc.vector.tensor_reduce(out=rsum[:], in_=r[:], axis=AxX,
                                            op=Alu.add)
                    junk = wp.tile([P, GA, HC], bf16, tag="junkA")
                    nc.scalar.activation(junk[:], r[:], Act.Square)
                    vsq = smp.tile([P, GA], f32, tag="vsq")
                    nc.vector.tensor_reduce(out=vsq[:], in_=junk[:], axis=AxX,
                                            op=Alu.add)
                    negmu = smp.tile([P, GA], f32, tag="negmu")
                    nc.vector.tensor_scalar(out=negmu[:], in0=rsum[:],
                                            scalar1=-1.0 / HC, scalar2=None,
                                            op0=Alu.mult)
                    t1 = smp.tile([P, GA], f32, tag="t1")
                    nc.vector.scalar_tensor_tensor(
                        out=t1[:], in0=rsum[:], scalar=1.0 / HC, in1=rsum[:],
                        op0=Alu.mult, op1=Alu.mult)
                    varHC = smp.tile([P, GA], f32, tag="varHC")
                    nc.vector.tensor_tensor(out=varHC[:], in0=vsq[:],
                                            in1=t1[:], op=Alu.subtract)
                    sd = smp.tile([P, GA], f32, tag="sd")
                    nc.scalar.activation(sd[:], varHC[:], Act.Sqrt,
                                         bias=eps_sb[:], scale=1.0 / HC)
                    rstd = smp.tile([P, GA], f32, tag="rstd")
                    nc.vector.reciprocal(rstd[:], sd[:])
                    cen = wp.tile([P, GA, HC], bf16, tag="cen")
                    nc.vector.tensor_tensor(
                        out=cen[:], in0=r[:],
                        in1=negmu[:].unsqueeze(2).to_broadcast([P, GA, HC]),
                        op=Alu.add)
                    psT = psT_pool.tile([P, GA, HC], bf16, tag="psT")
                    for i in range(GA):
                        nc.tensor.transpose(out=psT[:, i, :], in_=cen[:, i, :],
                                            identity=ident[:])
                    cenT = wp.tile([HC, GA, P], bf16, tag="cenT")
                    nc.scalar.activation(cenT[:], psT[:], Act.Copy)
                    psQ = psQ_pool.tile([P, GA, HID], f32, tag="psQ")
                    for i in range(GA):
                        nc.tensor.matmul(out=psQ[:, i, :], lhsT=cenT[:, i, :],
                                         rhs=wlint_sb[:], start=True, stop=True)
                    h_sb = wp.tile([P, GA, HC], bf16, tag="hsb")
                    nc.gpsimd.memset(h_sb[:, :, HID:HC], 0.0)
                    nc.vector.tensor_tensor(
                        out=h_sb[:, :, 0:HID], in0=psQ[:],
                        in1=rstd[:].unsqueeze(2).to_broadcast([P, GA, HID]),
                        op=Alu.mult)
                    n_ego = max(0, min(GA, NWIN - g0))
                    if n_ego > 0:
                        nc.vector.tensor_tensor(
                            out=ego_sb[:, g0 : g0 + n_ego, :],
                            in0=cen[:, 0:n_ego, :],
                            in1=rstd[:, 0:n_ego].unsqueeze(2).to_broadcast(
                                [P, n_ego, HC]),
                            op=Alu.mult)
                    nc.sync.dma_start(
                        hdram[g0 * P : (g0 + GA) * P, :].rearrange(
                            "(t p) f -> p t f", p=P),
                        h_sb[:])

                tc.strict_bb_all_engine_barrier()
                # ================= Phase B =================
                streams = {
                    "A": (colA_sb, rowA_sb, rdA_sb, hdram[0:SPLIT, :], SA, NCHA),
                    "B": (colB_sb, rowB_sb, rdB_sb, hdram[SPLIT:NP, :], SB, NCHB),
                }
                chunk_bufs = {"A": {}, "B": {}}

                def get_chunk(stream, c):
                    bufs = chunk_bufs[stream]
                    if c in bufs:
                        return bufs[c]
                    colsb, rowsb, _, hap, stot, _ = streams[stream]
                    n_i = min(CHUNK, stot - c * CHUNK)
                    nt = n_i // P
                    hc_b = gp.tile([P, CT, HC], bf16, tag="hc" + stream)
                    hr_b = gp.tile([P, CT, HC], bf16, tag="hr" + stream)
                    for h0 in range(0, n_i, CALL):
                        hn = min(CALL, n_i - h0)
                        i0 = c * (CHUNK // 16) + h0 // 16
                        i1 = i0 + (hn + 15) // 16
                        t0 = h0 // P
                        t1 = t0 + hn // P
                        nc.gpsimd.dma_gather(
                            hc_b[:, t0:t1, :], hap, colsb[:, i0:i1],
                            hn, hn, HC)
                        nc.gpsimd.dma_gather(
                            hr_b[:, t0:t1, :], hdram[0:SPLIT, :],
                            rowsb[:, i0:i1], hn, hn, HC)
                    # batched edge math over the chunk
                    tt = gp.tile([P, CT, HID], bf16, tag="tt" + stream)
                    nc.vector.scalar_tensor_tensor(
                        out=tt[:, :nt, :], in0=hr_b[:, :nt, 0:HID], scalar=0.5,
                        in1=hc_b[:, :nt, 0:HID], op0=Alu.mult, op1=Alu.add)
                    jk = gp.tile([P, CT, HID], bf16, tag="jk" + stream)
                    nc.vector.scalar_tensor_tensor(
                        out=jk[:, :nt, :], in0=tt[:, :nt, :], scalar=0.0,
                        in1=wd_sb[:].unsqueeze(1).to_broadcast([P, nt, HID]),
                        op0=Alu.max, op1=Alu.mult)
                    dd = gp.tile([P, CT], f32, tag="dd" + stream)
                    nc.vector.tensor_reduce(out=dd[:, :nt], in_=jk[:, :nt, :],
                                            axis=AxX, op=Alu.add)
                    att = gp.tile([P, CT], bf16, tag="at" + stream)
                    nc.scalar.activation(att[:, :nt], dd[:, :nt], Act.Sigmoid)
                    # xj into the gather buffer's pad half -> rhs = [hc | xj]
                    nc.vector.tensor_tensor(
                        out=hc_b[:, :nt, HID:HC], in0=hc_b[:, :nt, 0:HID],
                        in1=att[:, :nt].unsqueeze(2).to_broadcast([P, nt, HID]),
                        op=Alu.mult)
                    bufs[c] = hc_b
                    return hc_b

                gcnt = {"A": 0, "B": 0}
                for wi in range(NWIN):
                    ntile = T_A[wi] + T_B[wi]
                    if ntile == 0:
                        nc.vector.memset(agg_sb[:, wi, :], 0.0)
                        continue
                    acc = accp.tile([P, HC], f32, tag="acc")
                    ti = 0
                    for stream, tcount in (("A", T_A[wi]), ("B", T_B[wi])):
                        if tcount == 0:
                            continue
                        _, _, rdsb, _, _, _ = streams[stream]
                        g0 = gcnt[stream]
                        S_win = swp.tile([P, tcount, P], bf16, tag="Sw")
                        nc.vector.tensor_tensor(
                            out=S_win[:],
                            in0=iota_sb[:].unsqueeze(1).to_broadcast(
                                [P, tcount, P]),
                            in1=rdsb[:, g0 : g0 + tcount].unsqueeze(2)
                                .to_broadcast([P, tcount, P]),
                            op=Alu.is_equal)
                        for j in range(tcount):
                            g = g0 + j
                            buf = get_chunk(stream, g * P // CHUNK)
                            sub = (g * P % CHUNK) // P
                            nc.tensor.matmul(
                                out=acc[:], lhsT=S_win[:, j, :],
                                rhs=buf[:, sub, :],
                                start=(ti == 0), stop=(ti == ntile - 1))
                            ti += 1
                        gcnt[stream] += tcount
                    # agg = [xj_sum | hc_sum - xj_sum]
                    # (walrus rejects TT with two PSUM operands -> stage the
                    # xj half in SBUF first)
                    nc.scalar.activation(agg_sb[:, wi, 0:HID],
                                         acc[:, HID:HC], Act.Copy)
                    nc.vector.tensor_tensor(
                        out=agg_sb[:, wi, HID:HC], in0=acc[:, 0:HID],
                        in1=agg_sb[:, wi, 0:HID], op=Alu.subtract)

                # ================= Phase C =================
                for g in range(-(-NWIN // GC)):
                    g0 = g * GC
                    gg = min(GC, NWIN - g0)
                    xh = wp.tile([P, GC, HC], bf16, tag="xh")
                    nc.vector.tensor_scalar(out=xh[:, 0:gg, :],
                                            in0=agg_sb[:, g0:g0+gg, :],
                                            scalar1=0.0, scalar2=None,
                                            op0=Alu.max)
                    rsum = smp.tile([P, GC], f32, tag="rsumC")
                    nc.vector.tensor_reduce(out=rsum[:, 0:gg],
                                            in_=xh[:, 0:gg, :], axis=AxX,
                                            op=Alu.add)
                    junk = wp.tile([P, GC, HC], bf16, tag="junkC")
                    nc.scalar.activation(junk[:, 0:gg, :], xh[:, 0:gg, :],
                                         Act.Square)
                    vsq = smp.tile([P, GC], f32, tag="vsqC")
                    nc.vector.tensor_reduce(out=vsq[:, 0:gg],
                                            in_=junk[:, 0:gg, :], axis=AxX,
                                            op=Alu.add)
                    negmu = smp.tile([P, GC], f32, tag="negmuC")
                    nc.vector.tensor_scalar(out=negmu[:, 0:gg],
                                            in0=rsum[:, 0:gg],
                                            scalar1=-1.0 / HC, scalar2=None,
                                            op0=Alu.mult)
                    t1 = smp.tile([P, GC], f32, tag="t1C")
                    nc.vector.scalar_tensor_tensor(
                        out=t1[:, 0:gg], in0=rsum[:, 0:gg], scalar=1.0 / HC,
                        in1=rsum[:, 0:gg], op0=Alu.mult, op1=Alu.mult)
                    varHC = smp.tile([P, GC], f32, tag="varHCC")
                    nc.vector.tensor_tensor(out=varHC[:, 0:gg],
                                            in0=vsq[:, 0:gg],
                                            in1=t1[:, 0:gg], op=Alu.subtract)
                    sd = smp.tile([P, GC], f32, tag="sdC")
                    nc.scalar.activation(sd[:, 0:gg], varHC[:, 0:gg],
                                         Act.Sqrt, bias=eps_sb[:],
                                         scale=1.0 / HC)
                    rstd = smp.tile([P, GC], f32, tag="rstdC")
                    nc.vector.reciprocal(rstd[:, 0:gg], sd[:, 0:gg])
                    cen = wp.tile([P, GC, HC], bf16, tag="cenC")
                    nc.vector.tensor_tensor(
                        out=cen[:, 0:gg, :], in0=xh[:, 0:gg, :],
                        in1=negmu[:, 0:gg].unsqueeze(2).to_broadcast(
                            [P, gg, HC]),
                        op=Alu.add)
                    xbm = wp.tile([P, GC, HC], bf16, tag="xbm")
                    nc.vector.tensor_tensor(
                        out=xbm[:, 0:gg, :], in0=cen[:, 0:gg, :],
                        in1=rstd[:, 0:gg].unsqueeze(2).to_broadcast(
                            [P, gg, HC]),
                        op=Alu.mult)
                    xb = wp.tile([P, GC, HC], bf16, tag="xb")
                    nc.vector.tensor_tensor(
                        out=xb[:, 0:gg, :], in0=xbm[:, 0:gg, :],
                        in1=ego_sb[:, g0:g0+gg, :], op=Alu.add)
                    psT = psT_pool.tile([P, GA, HC], bf16, tag="psT")
                    for i in range(gg):
                        nc.tensor.transpose(out=psT[:, i, :], in_=xb[:, i, :],
                                            identity=ident[:])
                    xbT = wp.tile([HC, GC, P], bf16, tag="xbT")
                    nc.scalar.activation(xbT[:, 0:gg, :], psT[:, 0:gg, :],
                                         Act.Copy)
                    psO = psQ_pool.tile([P, GA, HID], f32, tag="psQ")
                    for i in range(gg):
                        nc.tensor.matmul(out=psO[:, i, 0:OUT],
                                         lhsT=xbT[:, i, :],
                                         rhs=w2t_sb[:], start=True, stop=False)
                        nc.tensor.matmul(out=psO[:, i, 0:OUT], lhsT=ones1[:],
                                         rhs=b2_sb[:], start=False, stop=True)
                    o_sb = wp.tile([P, GC, OUT], f32, tag="osb")
                    nc.scalar.activation(o_sb[:, 0:gg, :], psO[:, 0:gg, 0:OUT],
                                         Act.Copy)
                    nc.sync.dma_start(
                        outd[g0 * P : (g0 + gg) * P, :].rearrange(
                            "(t p) o -> p t o", p=P),
                        o_sb[:, 0:gg, :])
    nc.compile()
    return nc


def _get_compiled(key, T_A, T_B, reps):
    if key not in _cache:
        _cache[key] = _build(T_A, T_B, reps)
    return _cache[key]


def prepare(inputs, reps=1):
    """Host prep + build; returns (nc, in_maps)."""
    g0 = np.asarray(inputs["g0"])
    beta0 = np.asarray(inputs["beta0"])
    g1 = np.asarray(inputs["g1"])
    beta1 = np.asarray(inputs["beta1"])
    assert np.allclose(g0, 1.0) and np.allclose(beta0, 0.0), "LN affine"
    assert np.allclose(g1, 1.0) and np.allclose(beta1, 0.0), "LN affine"
    in_maps, (T_A, T_B) = _host_prep(
        inputs["x"], inputs["edge_index"], inputs["W1"], inputs["b1"],
        inputs["Wlin"], inputs["Watt"], inputs["W2"], inputs["b2"],
    )
    key = (T_A, T_B, reps)
    nc = _get_compiled(key, list(T_A), list(T_B), reps)
    return nc, in_maps


def kernel(**inputs) -> np.ndarray:
    from concourse.bass_utils import run_bass_kernel_spmd

    nc, in_maps = prepare(inputs, reps=1)
    res = run_bass_kernel_spmd(nc, in_maps, list(range(NCORES)))
    outs = [res.results[k]["out"] for k in range(NCORES)]
    full = np.concatenate(outs, axis=0)  # [NP, OUT] in global node order
    return full[:N]


# revision 19
# speedup vs baseline: 1.0438x; 1.0438x over previous
"""M2M-GNN (nn_M2MGNNPro) Trainium2 kernel, 8-core SPMD, v2.

Strategy (edge-parallel, destination-sharded, bf16 data path):
- Nodes padded to NP=50176, sharded 6272/core; per-core inputs rotated so the
  own shard occupies rows 0..6271 (identical SPMD program on all cores).
- Phase A (replicated): h0 = relu(x@W1.T+b1), ego = LN(h0), h = ego@Wlin.T
  written to a bf16 DRAM table with rows padded to 128 cols (256B for gather).
  Batched in groups of 4 node-tiles; LN stats via segmented reduces.
- Phase B (edge phase): edges sorted by dest window, split into col<32768 /
  col>=32768 gather streams (int16 idx). BOTH h[col] and h[row] fetched via
  gpsimd.dma_gather in CALL-slot chunks. Per chunk (batched across 6 tiles):
  tt = 0.5*hr+hc, d = wd.relu(tt) (seg-reduce), att = sigmoid(d), and
  xj = att*hc written into the gather buffer's pad half so each 128-edge tile
  scatters with ONE matmul: psum += S_t.T @ [hc | xj], S built per window by a
  single batched is_equal against iota. agg = [xj_sum | hc_sum - xj_sum].
- Phase C: relu/LN/blend (0.5 folded into W2) + GEMM, batched 7 tiles/group.
"""
import numpy as np

N = 50000
E = 800000
IN = 128
HID = 64
C = 2
HC = 128
OUT = 40
BETA = 0.5
TEMP = 1.0
EPS = 1e-5

NCORES = 8
P = 128
NP = 50176            # 392 tiles of 128
SH = NP // NCORES     # 6272 nodes/core, 49 windows
NWIN = SH // P        # 49
NT = NP // P          # 392
SPLIT = 32768         # int16-safe col split
CALL = 768            # gather slots per dma_gather call (ring-safe)
CHUNK = 1536          # slots per batched-math chunk (2 gather calls)
CT = CHUNK // P       # tiles per chunk
GA = 4                # phase A tiles per group
GC = 4                # phase C tiles per group

_cache = {}


def _bf16():
    import concourse.mybir as mybir
    return mybir.dt.np(mybir.dt.bfloat16)


def _host_prep(x, edge_index, W1, b1, Wlin, Watt, W2, b2):
    bf16 = _bf16()
    x = np.asarray(x, np.float32)
    row = np.asarray(edge_index[0], np.int64)
    col = np.asarray(edge_index[1], np.int64)

    x_pad = np.zeros((NP, IN), np.float32)
    x_pad[:N] = x

    core = row // SH
    meta_TA = np.zeros(NWIN, np.int64)
    meta_TB = np.zeros(NWIN, np.int64)
    percore = []
    for k in range(NCORES):
        m = core == k
        rk = row[m] - k * SH          # local dest 0..SH-1
        ck = (col[m] - k * SH) % NP   # rotated col index
        w = rk // P
        groups = []
        for wi in range(NWIN):
            mw = w == wi
            cw, rw, rl = ck[mw], rk[mw] % P, rk[mw]
            a = cw < SPLIT
            groups.append(
                ((cw[a], rw[a], rl[a]), (cw[~a] - SPLIT, rw[~a], rl[~a]))
            )
        percore.append(groups)
    for wi in range(NWIN):
        meta_TA[wi] = max(
            -(-len(percore[k][wi][0][0]) // P) for k in range(NCORES)
        )
        meta_TB[wi] = max(
            -(-len(percore[k][wi][1][0]) // P) for k in range(NCORES)
        )
    T_A, T_B = meta_TA, meta_TB
    SA = int(T_A.sum()) * P
    SB = int(T_B.sum()) * P

    def wrap16(a):
        n = len(a)
        pad = (-n) % 16
        a = np.concatenate([a, np.zeros(pad, np.int16)])
        return np.tile(a.reshape(-1, 16).T, (8, 1))

    def tilecols(a):
        # slot i=(t*128+p) -> [128, ntiles] col-per-tile layout
        return a.reshape(-1, P).T.copy()

    in_maps = []
    for k in range(NCORES):
        colA = np.zeros(SA, np.int16)
        rowA = np.zeros(SA, np.int16)
        rdA = np.full(SA, 200.0, np.float32)
        colB = np.zeros(SB, np.int16)
        rowB = np.zeros(SB, np.int16)
        rdB = np.full(SB, 200.0, np.float32)
        oa = ob = 0
        for wi in range(NWIN):
            (ca, ra, la), (cb, rb, lb) = percore[k][wi]
            na, nb = len(ca), len(cb)
            colA[oa : oa + na] = ca.astype(np.int16)
            rowA[oa : oa + na] = la.astype(np.int16)
            rdA[oa : oa + na] = ra.astype(np.float32)
            colB[ob : ob + nb] = cb.astype(np.int16)
            rowB[ob : ob + nb] = lb.astype(np.int16)
            rdB[ob : ob + nb] = rb.astype(np.float32)
            oa += int(T_A[wi]) * P
            ob += int(T_B[wi]) * P

        xk = np.roll(x_pad, -k * SH, axis=0)
        in_maps.append(
            {
                "xT": xk.T.astype(bf16).copy(),
                "colA": wrap16(colA),
                "colB": wrap16(colB),
                "rdA": tilecols(rdA).astype(bf16),
                "rdB": tilecols(rdB).astype(bf16),
            }
        )
    wd = (np.asarray(Watt[0]) - np.asarray(Watt[1])).astype(np.float32)
    shared = {
        "w1t": np.asarray(W1, np.float32).T.astype(bf16).copy(),   # [IN, HC]
        "b1row": np.asarray(b1, np.float32)[None, :].astype(bf16), # [1, HC]
        "wlint": np.asarray(Wlin, np.float32).T.astype(bf16).copy(),  # [HC, HID]
        "wdrep": np.tile(wd[None, :], (P, 1)).astype(bf16),        # [P, HID]
        "iotac": np.tile(
            np.arange(P, dtype=np.float32)[None, :], (P, 1)
        ).astype(bf16),                                            # [P, P]
        "w2t": ((1.0 - BETA) * np.asarray(W2, np.float32).T).astype(bf16).copy(),
        "b2row": np.asarray(b2, np.float32)[None, :].astype(bf16), # [1, OUT]
    }
    for im in in_maps:
        im.update(shared)
    return in_maps, (tuple(T_A.tolist()), tuple(T_B.tolist()))


def _build(T_A, T_B, reps=1):
    import concourse.bacc as bacc
    import concourse.mybir as mybir
    import concourse.tile as tile
    from concourse.library_config import mlp
    from concourse.masks import make_identity

    f32 = mybir.dt.float32
    bf16 = mybir.dt.bfloat16
    i16 = mybir.dt.int16
    Alu = mybir.AluOpType
    Act = mybir.ActivationFunctionType
    AxX = mybir.AxisListType.X

    SA = sum(T_A) * P
    SB = sum(T_B) * P
    NCHA = -(-SA // CHUNK)
    NCHB = -(-SB // CHUNK)

    nc = bacc.Bacc("TRN2")
    xT = nc.dram_tensor("xT", [IN, NP], bf16, kind="ExternalInput")
    colA = nc.dram_tensor("colA", [P, (SA + 15) // 16], i16, kind="ExternalInput")
    colB = nc.dram_tensor("colB", [P, (SB + 15) // 16], i16, kind="ExternalInput")
    rdA = nc.dram_tensor("rdA", [P, SA // P], bf16, kind="ExternalInput")
    rdB = nc.dram_tensor("rdB", [P, SB // P], bf16, kind="ExternalInput")
    w1t = nc.dram_tensor("w1t", [IN, HC], bf16, kind="ExternalInput")
    b1row = nc.dram_tensor("b1row", [1, HC], bf16, kind="ExternalInput")
    wlint = nc.dram_tensor("wlint", [HC, HID], bf16, kind="ExternalInput")
    wdrep = nc.dram_tensor("wdrep", [P, HID], bf16, kind="ExternalInput")
    iotac = nc.dram_tensor("iotac", [P, P], bf16, kind="ExternalInput")
    w2t = nc.dram_tensor("w2t", [HC, OUT], bf16, kind="ExternalInput")
    b2row = nc.dram_tensor("b2row", [1, OUT], bf16, kind="ExternalInput")
    hdram = nc.dram_tensor("hdram", [NP, HC], bf16)
    outd = nc.dram_tensor("out", [SH, OUT], f32, kind="ExternalOutput")

    with tile.TileContext(nc) as tc:
        with (
            tc.tile_pool(name="const", bufs=1) as cp,
            tc.tile_pool(name="work", bufs=3) as wp,
            tc.tile_pool(name="sm", bufs=3) as smp,
            tc.tile_pool(name="gather", bufs=4) as gp,
            tc.tile_pool(name="swin", bufs=3) as swp,
            tc.tile_pool(name="psA", bufs=2, space="PSUM") as psA_pool,
            tc.tile_pool(name="psT", bufs=2, space="PSUM") as psT_pool,
            tc.tile_pool(name="psQ", bufs=1, space="PSUM") as psQ_pool,
            tc.tile_pool(name="psR", bufs=1, space="PSUM") as psR_pool,
            tc.tile_pool(name="acc", bufs=2, space="PSUM") as accp,
        ):
            nc.gpsimd.load_library(mlp)
            # ---- constants to SBUF ----
            w1t_sb = cp.tile([IN, HC], bf16, tag="w1t")
            b1_sb = cp.tile([1, HC], bf16, tag="b1")
            wlint_sb = cp.tile([HC, HID], bf16, tag="wlt")
            wd_sb = cp.tile([P, HID], bf16, tag="wd")
            iota_sb = cp.tile([P, P], bf16, tag="iota")
            w2t_sb = cp.tile([HC, OUT], bf16, tag="w2t")
            b2_sb = cp.tile([1, OUT], bf16, tag="b2")
            colA_sb = cp.tile([P, (SA + 15) // 16], i16, tag="colA")
            colB_sb = cp.tile([P, (SB + 15) // 16], i16, tag="colB")
            rdA_sb = cp.tile([P, SA // P], bf16, tag="rdA")
            rdB_sb = cp.tile([P, SB // P], bf16, tag="rdB")
            for sb, dr in (
                (w1t_sb, w1t), (b1_sb, b1row), (wlint_sb, wlint),
                (wd_sb, wdrep), (iota_sb, iotac), (w2t_sb, w2t),
                (b2_sb, b2row), (colA_sb, colA), (colB_sb, colB),
                (rdA_sb, rdA), (rdB_sb, rdB),
            ):
                nc.sync.dma_start(sb[:], dr[:])
            ident = cp.tile([P, P], bf16, tag="ident")
            make_identity(nc, ident[:])
            ones1 = cp.tile([1, P], bf16, tag="ones1")
            nc.vector.memset(ones1[:], 1.0)
            eps_sb = cp.tile([P, 1], f32, tag="eps")
            nc.vector.memset(eps_sb[:], EPS)
            ego_sb = cp.tile([P, NWIN, HC], bf16, tag="ego")
            agg_sb = cp.tile([P, NWIN, HC], bf16, tag="agg")

            for rep in range(reps):
                tc.strict_bb_all_engine_barrier()
                # ================= Phase A =================
                for g in range(NT // GA):
                    g0 = g * GA
                    xt_t = wp.tile([IN, GA * P], bf16, tag="xt")
                    nc.sync.dma_start(xt_t[:], xT[:, g0 * P : (g0 + GA) * P])
                    psA = psA_pool.tile([P, GA, HC], f32, tag="psA")
                    for i in range(GA):
                        nc.tensor.matmul(out=psA[:, i, :],
                                         lhsT=xt_t[:, i * P : (i + 1) * P],
                                         rhs=w1t_sb[:], start=True, stop=False)
                        nc.tensor.matmul(out=psA[:, i, :], lhsT=ones1[:],
                                         rhs=b1_sb[:], start=False, stop=True)
                    r = wp.tile([P, GA, HC], bf16, tag="r")
                    nc.scalar.activation(r[:], psA[:], Act.Relu)
                    rsum = smp.tile([P, GA], f32, tag="rsum")
                    nc.vector.tensor_reduce(out=rsum[:], in_=r[:], axis=AxX,
                                            op=Alu.add)
                    junk = wp.tile([P, GA, HC], bf16, tag="junkA")
                    nc.scalar.activation(junk[:], r[:], Act.Square)
                    vsq = smp.tile([P, GA], f32, tag="vsq")
                    nc.vector.tensor_reduce(out=vsq[:], in_=junk[:], axis=AxX,
                                            op=Alu.add)
                    negmu = smp.tile([P, GA], f32, tag="negmu")
                    nc.vector.tensor_scalar(out=negmu[:], in0=rsum[:],
                                            scalar1=-1.0 / HC, scalar2=None,
                                            op0=Alu.mult)
                    t1 = smp.tile([P, GA], f32, tag="t1")
                    nc.vector.scalar_tensor_tensor(
                        out=t1[:], in0=rsum[:], scalar=1.0 / HC, in1=rsum[:],
                        op0=Alu.mult, op1=Alu.mult)
                    varHC = smp.tile([P, GA], f32, tag="varHC")
                    nc.vector.tensor_tensor(out=varHC[:], in0=vsq[:],
                                            in1=t1[:], op=Alu.subtract)
                    sd = smp.tile([P, GA], f32, tag="sd")
                    nc.scalar.activation(sd[:], varHC[:], Act.Sqrt,
                                         bias=eps_sb[:], scale=1.0 / HC)
                    rstd = smp.tile([P, GA], f32, tag="rstd")
                    nc.vector.reciprocal(rstd[:], sd[:])
                    cen = wp.tile([P, GA, HC], bf16, tag="cen")
                    nc.vector.tensor_tensor(
                        out=cen[:], in0=r[:],
                        in1=negmu[:].unsqueeze(2).to_broadcast([P, GA, HC]),
                        op=Alu.add)
                    psT = psT_pool.tile([P, GA, HC], bf16, tag="psT")
                    for i in range(GA):
                        nc.tensor.transpose(out=psT[:, i, :], in_=cen[:, i, :],
                                            identity=ident[:])
                    cenT = wp.tile([HC, GA, P], bf16, tag="cenT")
                    nc.scalar.activation(cenT[:], psT[:], Act.Copy)
                    psQ = psQ_pool.tile([P, GA, HID], f32, tag="psQ")
                    for i in range(GA):
                        nc.tensor.matmul(out=psQ[:, i, :], lhsT=cenT[:, i, :],
                                         rhs=wlint_sb[:], start=True, stop=True)
                    h_sb = wp.tile([P, GA, HC], bf16, tag="hsb")
                    nc.gpsimd.memset(h_sb[:, :, HID:HC], 0.0)
                    nc.vector.tensor_tensor(
                        out=h_sb[:, :, 0:HID], in0=psQ[:],
                        in1=rstd[:].unsqueeze(2).to_broadcast([P, GA, HID]),
                        op=Alu.mult)
                    n_ego = max(0, min(GA, NWIN - g0))
                    if n_ego > 0:
                        nc.vector.tensor_tensor(
                            out=ego_sb[:, g0 : g0 + n_ego, :],
                            in0=cen[:, 0:n_ego, :],
                            in1=rstd[:, 0:n_ego].unsqueeze(2).to_broadcast(
                                [P, n_ego, HC]),
                            op=Alu.mult)
                    nc.sync.dma_start(
                        hdram[g0 * P : (g0 + GA) * P, :].rearrange(
                            "(t p) f -> p t f", p=P),
                        h_sb[:])

                tc.strict_bb_all_engine_barrier()
                # ================= Phase B =================
                streams = {
                    "A": (colA_sb, rdA_sb, hdram[0:SPLIT, :], SA, NCHA),
                    "B": (colB_sb, rdB_sb, hdram[SPLIT:NP, :], SB, NCHB),
                }
                chunk_bufs = {"A": {}, "B": {}}

                def get_chunk(stream, c):
                    bufs = chunk_bufs[stream]
                    if c in bufs:
                        return bufs[c]
                    colsb, _, hap, stot, _ = streams[stream]
                    n_i = min(CHUNK, stot - c * CHUNK)
                    hc_b = gp.tile([P, CT, HC], bf16, tag="hc" + stream)
                    for h0 in range(0, n_i, CALL):
                        hn = min(CALL, n_i - h0)
                        i0 = c * (CHUNK // 16) + h0 // 16
                        i1 = i0 + (hn + 15) // 16
                        t0 = h0 // P
                        t1 = t0 + hn // P
                        nc.gpsimd.dma_gather(
                            hc_b[:, t0:t1, :], hap, colsb[:, i0:i1],
                            hn, hn, HC)
                    bufs[c] = hc_b
                    return hc_b

                gcnt = {"A": 0, "B": 0}
                for wi in range(NWIN):
                    ntile = T_A[wi] + T_B[wi]
                    if ntile == 0:
                        nc.vector.memset(agg_sb[:, wi, :], 0.0)
                        continue
                    hwin = swp.tile([P, HID], bf16, tag="hwin")
                    nc.sync.dma_start(
                        hwin[:], hdram[wi * P : (wi + 1) * P, 0:HID])
                    acc = accp.tile([P, HC], f32, tag="acc")
                    ti = 0
                    for stream, tcount in (("A", T_A[wi]), ("B", T_B[wi])):
                        if tcount == 0:
                            continue
                        _, rdsb, _, _, _ = streams[stream]
                        g0 = gcnt[stream]
                        S_win = swp.tile([P, tcount, P], bf16, tag="Sw")
                        nc.vector.tensor_tensor(
                            out=S_win[:],
                            in0=iota_sb[:].unsqueeze(1).to_broadcast(
                                [P, tcount, P]),
                            in1=rdsb[:, g0 : g0 + tcount].unsqueeze(2)
                                .to_broadcast([P, tcount, P]),
                            op=Alu.is_equal)
                        # expand hr = S @ hwin per tile (PE), batched copies
                        psS = psT_pool.tile([P, GA, HC], bf16, tag="psT")
                        STw = swp.tile([P, tcount, P], bf16, tag="STw")
                        for j in range(tcount):
                            nc.tensor.transpose(out=psS[:, j % GA, :],
                                                in_=S_win[:, j, :],
                                                identity=ident[:])
                            if j % GA == GA - 1 or j == tcount - 1:
                                jlo = (j // GA) * GA
                                nc.scalar.activation(
                                    STw[:, jlo : j + 1, :],
                                    psS[:, 0 : j + 1 - jlo, :], Act.Copy)
                                psS = psT_pool.tile([P, GA, HC], bf16,
                                                    tag="psT")
                        psR = psR_pool.tile([P, CT // 2, HID], f32, tag="psR")
                        hr_sb = gp.tile([P, tcount, HID], bf16,
                                        tag="hr" + stream)
                        for j in range(tcount):
                            nc.tensor.matmul(
                                out=psR[:, j % (CT // 2), :],
                                lhsT=STw[:, j, :], rhs=hwin[:],
                                start=True, stop=True)
                            if j % (CT // 2) == CT // 2 - 1 or j == tcount - 1:
                                jlo = (j // (CT // 2)) * (CT // 2)
                                nc.vector.tensor_copy(
                                    hr_sb[:, jlo : j + 1, :],
                                    psR[:, 0 : j + 1 - jlo, :])
                                psR = psR_pool.tile([P, CT // 2, HID], f32,
                                                    tag="psR")
                        # batched edge math over the window's tiles
                        tt = gp.tile([P, CT, HID], bf16, tag="tt" + stream)
                        first = get_chunk(stream, g0 * P // CHUNK)
                        segs = []
                        j = 0
                        while j < tcount:
                            g = g0 + j
                            c = g * P // CHUNK
                            sub = (g * P % CHUNK) // P
                            npart = min(tcount - j, CT - sub)
                            segs.append((j, get_chunk(stream, c), sub, npart))
                            j += npart
                        for (j0, buf, sub, npart) in segs:
                            nc.vector.scalar_tensor_tensor(
                                out=tt[:, j0 : j0 + npart, :],
                                in0=hr_sb[:, j0 : j0 + npart, :], scalar=0.5,
                                in1=buf[:, sub : sub + npart, 0:HID],
                                op0=Alu.mult, op1=Alu.add)
                        jkw = gp.tile([P, CT, HID], bf16, tag="jk" + stream)
                        nc.vector.scalar_tensor_tensor(
                            out=jkw[:, 0:tcount, :], in0=tt[:, 0:tcount, :],
                            scalar=0.0,
                            in1=wd_sb[:].unsqueeze(1).to_broadcast(
                                [P, tcount, HID]),
                            op0=Alu.max, op1=Alu.mult)
                        dd = gp.tile([P, CT], f32, tag="dd" + stream)
                        nc.vector.tensor_reduce(out=dd[:, 0:tcount],
                                                in_=jkw[:, 0:tcount, :],
                                                axis=AxX, op=Alu.add)
                        att = gp.tile([P, CT], bf16, tag="at" + stream)
                        nc.scalar.activation(att[:, 0:tcount], dd[:, 0:tcount],
                                             Act.Sigmoid)
                        for (j0, buf, sub, npart) in segs:
                            nc.vector.tensor_tensor(
                                out=buf[:, sub : sub + npart, HID:HC],
                                in0=buf[:, sub : sub + npart, 0:HID],
                                in1=att[:, j0 : j0 + npart].unsqueeze(2)
                                    .to_broadcast([P, npart, HID]),
                                op=Alu.mult)
                        for j in range(tcount):
                            g = g0 + j
                            buf = chunk_bufs[stream][g * P // CHUNK]
                            sub = (g * P % CHUNK) // P
                            nc.tensor.matmul(
                                out=acc[:], lhsT=S_win[:, j, :],
                                rhs=buf[:, sub, :],
                                start=(ti == 0), stop=(ti == ntile - 1))
                            ti += 1
                        gcnt[stream] += tcount
                    # agg = [xj_sum | hc_sum - xj_sum]
                    # (walrus rejects TT with two PSUM operands -> stage the
                    # xj half in SBUF first)
                    nc.scalar.activation(agg_sb[:, wi, 0:HID],
                                         acc[:, HID:HC], Act.Copy)
                    nc.vector.tensor_tensor(
                        out=agg_sb[:, wi, HID:HC], in0=acc[:, 0:HID],
                        in1=agg_sb[:, wi, 0:HID], op=Alu.subtract)

                # ================= Phase C =================
                for g in range(-(-NWIN // GC)):
                    g0 = g * GC
                    gg = min(GC, NWIN - g0)
                    xh = wp.tile([P, GC, HC], bf16, tag="xh")
                    nc.vector.tensor_scalar(out=xh[:, 0:gg, :],
                                            in0=agg_sb[:, g0:g0+gg, :],
                                            scalar1=0.0, scalar2=None,
                                            op0=Alu.max)
                    rsum = smp.tile([P, GC], f32, tag="rsumC")
                    nc.vector.tensor_reduce(out=rsum[:, 0:gg],
                                            in_=xh[:, 0:gg, :], axis=AxX,
                                            op=Alu.add)
                    junk = wp.tile([P, GC, HC], bf16, tag="junkC")
                    nc.scalar.activation(junk[:, 0:gg, :], xh[:, 0:gg, :],
                                         Act.Square)
                    vsq = smp.tile([P, GC], f32, tag="vsqC")
                    nc.vector.tensor_reduce(out=vsq[:, 0:gg],
                                            in_=junk[:, 0:gg, :], axis=AxX,
                                            op=Alu.add)
                    negmu = smp.tile([P, GC], f32, tag="negmuC")
                    nc.vector.tensor_scalar(out=negmu[:, 0:gg],
                                            in0=rsum[:, 0:gg],
                                            scalar1=-1.0 / HC, scalar2=None,
                                            op0=Alu.mult)
                    t1 = smp.tile([P, GC], f32, tag="t1C")
                    nc.vector.scalar_tensor_tensor(
                        out=t1[:, 0:gg], in0=rsum[:, 0:gg], scalar=1.0 / HC,
                        in1=rsum[:, 0:gg], op0=Alu.mult, op1=Alu.mult)
                    varHC = smp.tile([P, GC], f32, tag="varHCC")
                    nc.vector.tensor_tensor(out=varHC[:, 0:gg],
                                            in0=vsq[:, 0:gg],
                                            in1=t1[:, 0:gg], op=Alu.subtract)
                    sd = smp.tile([P, GC], f32, tag="sdC")
                    nc.scalar.activation(sd[:, 0:gg], varHC[:, 0:gg],
                                         Act.Sqrt, bias=eps_sb[:],
                                         scale=1.0 / HC)
                    rstd = smp.tile([P, GC], f32, tag="rstdC")
                    nc.vector.reciprocal(rstd[:, 0:gg], sd[:, 0:gg])
                    cen = wp.tile([P, GC, HC], bf16, tag="cenC")
                    nc.vector.tensor_tensor(
                        out=cen[:, 0:gg, :], in0=xh[:, 0:gg, :],
                        in1=negmu[:, 0:gg].unsqueeze(2).to_broadcast(
                            [P, gg, HC]),
                        op=Alu.add)
                    xbm = wp.tile([P, GC, HC], bf16, tag="xbm")
                    nc.vector.tensor_tensor(
                        out=xbm[:, 0:gg, :], in0=cen[:, 0:gg, :],
                        in1=rstd[:, 0:gg].unsqueeze(2).to_broadcast(
                            [P, gg, HC]),
                        op=Alu.mult)
                    xb = wp.tile([P, GC, HC], bf16, tag="xb")
                    nc.vector.tensor_tensor(
                        out=xb[:, 0:gg, :], in0=xbm[:, 0:gg, :],
                        in1=ego_sb[:, g0:g0+gg, :], op=Alu.add)
                    psT = psT_pool.tile([P, GA, HC], bf16, tag="psT")
                    for i in range(gg):
                        nc.tensor.transpose(out=psT[:, i, :], in_=xb[:, i, :],
                                            identity=ident[:])
                    xbT = wp.tile([HC, GC, P], bf16, tag="xbT")
                    nc.scalar.activation(xbT[:, 0:gg, :], psT[:, 0:gg, :],
                                         Act.Copy)
                    psO = psQ_pool.tile([P, GA, HID], f32, tag="psQ")
                    for i in range(gg):
                        nc.tensor.matmul(out=psO[:, i, 0:OUT],
                                         lhsT=xbT[:, i, :],
                                         rhs=w2t_sb[:], start=True, stop=False)
                        nc.tensor.matmul(out=psO[:, i, 0:OUT], lhsT=ones1[:],
                                         rhs=b2_sb[:], start=False, stop=True)
                    o_sb = wp.tile([P, GC, OUT], f32, tag="osb")
                    nc.scalar.activation(o_sb[:, 0:gg, :], psO[:, 0:gg, 0:OUT],
                                         Act.Copy)
                    nc.sync.dma_start(
                        outd[g0 * P : (g0 + gg) * P, :].rearrange(
                            "(t p) o -> p t o", p=P),
                        o_sb[:, 0:gg, :])
    nc.compile()
    return nc


def _get_compiled(key, T_A, T_B, reps):
    if key not in _cache:
        _cache[key] = _build(T_A, T_B, reps)
    return _cache[key]


def prepare(inputs, reps=1):
    """Host prep + build; returns (nc, in_maps)."""
    g0 = np.asarray(inputs["g0"])
    beta0 = np.asarray(inputs["beta0"])
    g1 = np.asarray(inputs["g1"])
    beta1 = np.asarray(inputs["beta1"])
    assert np.allclose(g0, 1.0) and np.allclose(beta0, 0.0), "LN affine"
    assert np.allclose(g1, 1.0) and np.allclose(beta1, 0.0), "LN affine"
    in_maps, (T_A, T_B) = _host_prep(
        inputs["x"], inputs["edge_index"], inputs["W1"], inputs["b1"],
        inputs["Wlin"], inputs["Watt"], inputs["W2"], inputs["b2"],
    )
    key = (T_A, T_B, reps)
    nc = _get_compiled(key, list(T_A), list(T_B), reps)
    return nc, in_maps


def kernel(**inputs) -> np.ndarray:
    from concourse.bass_utils import run_bass_kernel_spmd

    nc, in_maps = prepare(inputs, reps=1)
    res = run_bass_kernel_spmd(nc, in_maps, list(range(NCORES)))
    outs = [res.results[k]["out"] for k in range(NCORES)]
    full = np.concatenate(outs, axis=0)  # [NP, OUT] in global node order
    return full[:N]
